# revision 26
# baseline (speedup 1.0000x reference)
"""Causal multi-head self-attention on 8 Trainium2 NeuronCores.

Problem: x[4,2048,1024], Wq/Wk/Wv/Wo[1024,1024], H=16 heads, dk=64.
  q = x@Wq.T, k = x@Wk.T, v = x@Wv.T  (per-head causal softmax(q k^T/8) v) @ Wo.T

Sharding: core c handles batch b=c//2 and head-half hh=c%2 (8 heads).
Each core returns a partial output (its 512 attn columns through the
matching 512 rows of Wo.T); the host sums core pairs.

Kernel layouts (all contractions on the partition axis):
  xT  [1024, 2048]  (d, s)      qT/kT [512, 2048] (head*64+dk, s)
  v   [s-block, head, 65]       (col 64 = ones -> softmax denominator)
  scoresT [k, q] blocks; exp on ACT (scale=1/8, no max-subtraction --
  scores are O(1) here); causal = block skip + affine_select zero-fill
  on diagonal blocks; attnT accumulated in PSUM with the ones column
  giving the denominator; normalization via DVE reciprocal + a rank-1
  PE broadcast matmul; O-projection from attnT layout.
Matmul inputs are bitcast to float32r (full-rate on the PE array).
"""

import numpy as np

import concourse.bass as bass
import concourse.mybir as mybir
import concourse.tile as tile
from concourse.bass_utils import run_bass_kernel_spmd
from concourse.vector_clock import ScopedClock, VectorClock

B, S, D, H, DK = 4, 2048, 1024, 16, 64
HPC = H // 2          # heads per core
HD = HPC * DK         # 512 head-dim columns per core
CH = 512              # q-chunk width
NCH = S // CH         # 4
NKB = S // 128        # 16 k-blocks
F32 = mybir.dt.float32
F32R = mybir.dt.float32r
EXP = mybir.ActivationFunctionType.Exp


def _drain_and_barrier_split(self, tick_clock, wait_clock):
    # The stock Tile tail drain attaches every outstanding sem wait to one
    # Drain instruction; this walrus build caps sync waits per instruction
    # and rejects it.  Put each wait on its own SP nop first, then drain
    # with no waits (SP has observed everything by then).
    gc = tick_clock.global_clock
    n = len(gc)
    for proc in range(n):
        t = gc[proc]
        if t == 0:
            continue
        vc = VectorClock([0] * n)
        vc.require_at_least(proc, t)
        nop = self.nc.sync.nop(nofuse=True)
        wait_clock.add_sem_waits(nop.ins, ScopedClock({None: vc}))
    self.nc.sync.drain()
    self.nc.all_engine_barrier()
    assert self.sems is not None
    popped = self.nc._tile_sem_poison_stack.pop()
    assert popped is self._sem_poison
    self.nc.clear_and_free_semaphores(list(self.sems.allocated().values()))
    self.nc.all_engine_barrier()


def _build_kernel(ctx, tc, xT, wqT, wkT, wvT, woT, out):
    nc = tc.nc
    KC = D // 128  # 8 contraction chunks for the projections

    wpool = ctx.enter_context(tc.tile_pool(name="weights", bufs=1))
    kvpool = ctx.enter_context(tc.tile_pool(name="kv", bufs=1))
    xpool = ctx.enter_context(tc.tile_pool(name="x", bufs=1))
    qpool = ctx.enter_context(tc.tile_pool(name="q", bufs=2))
    epool = ctx.enter_context(tc.tile_pool(name="exp", bufs=4))
    apool = ctx.enter_context(tc.tile_pool(name="attn", bufs=2))
    opool = ctx.enter_context(tc.tile_pool(name="osb", bufs=4))
    rpool = ctx.enter_context(tc.tile_pool(name="recip", bufs=2))
    # One PSUM pool, 8 banks: sc 2x[128,1024] (4) + at 3x[65,512] (3) +
    # bc 1x[64,512] (1).  Projection/O-proj groups share the "sc" slots.
    pp = ctx.enter_context(tc.tile_pool(name="pp", bufs=2, space="PSUM"))

    # --- whole-kernel-resident tiles ---
    wq = [wpool.tile([128, HD], F32R, tag=f"wq{kc}", name=f"wq{kc}")
          for kc in range(KC)]
    wk = [wpool.tile([128, HD], F32R, tag=f"wk{kc}", name=f"wk{kc}")
          for kc in range(KC)]
    wv = [wpool.tile([128, HD], F32R, tag=f"wv{kc}", name=f"wv{kc}")
          for kc in range(KC)]
    wo = wpool.tile([128, 4, D], F32R, tag="wo")
    ones = wpool.tile([1, DK], F32R, tag="ones")
    kT = kvpool.tile([128, 4, S], F32R, tag="kT")
    v = kvpool.tile([128, NKB, HPC, DK + 1], F32R, tag="v")

    def dma_x(j):
        cs = slice(j * CH, (j + 1) * CH)
        xch = [xpool.tile([128, CH], F32R, tag=f"x{kc}", name=f"x{kc}")
               for kc in range(KC)]
        for kc in range(KC):
            nc.sync.dma_start(out=xch[kc], in_=xT[kc * 128:(kc + 1) * 128, cs])
        return xch

    def dma_w(w, wT):
        for kc in range(KC):
            nc.sync.dma_start(out=w[kc], in_=wT[kc * 128:(kc + 1) * 128, :])

    # First matmul needs only xch0[0] + wq[0]: interleave those DMAs first.
    cs0 = slice(0, CH)
    xch0 = [xpool.tile([128, CH], F32R, tag=f"x{kc}", name=f"x{kc}")
            for kc in range(KC)]
    for kc in range(KC):
        nc.sync.dma_start(out=xch0[kc], in_=xT[kc * 128:(kc + 1) * 128, cs0])
        nc.gpsimd.dma_start(out=wq[kc], in_=wqT[kc * 128:(kc + 1) * 128, :])
    dma_w(wk, wkT)
    dma_w(wv, wvT)
    nc.sync.dma_start(out=wo, in_=woT.rearrange("(c p) n -> p c n", p=128))
    ones_f32 = wpool.tile([1, DK], F32, tag="ones_f32")
    nc.vector.memset(ones_f32, 1.0)
    nc.vector.tensor_copy(ones, ones_f32)
    vcol_f32 = wpool.tile([128, NKB, HPC, 1], F32, tag="vcol_f32")
    nc.vector.memset(vcol_f32, 1.0)
    nc.vector.tensor_copy(v[:, :, :, DK:DK + 1], vcol_f32)

    def qkv_fillers(j, xch):
        cs = slice(j * CH, (j + 1) * CH)
        qch = qpool.tile([128, 4, CH], F32R, name=f"qch{j}", tag="qch")
        fillers = []
        dense = j == 0  # attention not running yet: borrow the sc slots

        def proj(w, dst, mb, nmb):
            def f():
                tag = "sc" if dense else "fill"
                ps = pp.tile([128, nmb * CH], F32, tag=tag, bufs=None if dense
                             else 1, name="psf")
                for t in range(nmb):
                    for kc in range(KC):
                        nc.tensor.matmul(
                            ps[:, t * CH:(t + 1) * CH],
                            lhsT=w[kc][:, (mb + t) * 128:(mb + t + 1) * 128],
                            rhs=xch[kc], start=(kc == 0), stop=(kc == KC - 1))
                nc.vector.tensor_copy(dst, ps)
            return f

        def vproj(sb, nsb):
            def f():
                tag = "sc" if dense else "fill"
                ps = pp.tile([128, nsb * CH], F32, tag=tag, bufs=None if dense
                             else 1, name="psf")
                for t in range(nsb):
                    for kc in range(KC):
                        nc.tensor.matmul(
                            ps[:, t * CH:(t + 1) * CH],
                            lhsT=xch[kc][:, (sb + t) * 128:(sb + t + 1) * 128],
                            rhs=wv[kc], start=(kc == 0), stop=(kc == KC - 1))
                sblk = j * 4 + sb
                nc.vector.tensor_copy(
                    v[:, sblk:sblk + nsb, :, 0:DK],
                    ps.rearrange("p (t h d) -> p t h d", t=nsb, h=HPC))
            return f

        nm = 2 if dense else 1
        for mb in range(0, 4, nm):
            fillers.append(proj(wq, qch[:, mb:mb + nm, :], mb, nm))
        for mb in range(0, 4, nm):
            fillers.append(proj(wk, kT[:, mb:mb + nm, cs], mb, nm))
        for sb in range(0, 4, nm):
            fillers.append(vproj(sb, nm))
        return qch, fillers

    def o_fillers(j, ach):
        def oblk(sb, n):
            def f():
                sblk = j * 4 + sb
                osb = opool.tile([128, CH], F32, name="osb", tag="osb")
                ps = pp.tile([128, CH], F32, tag="fill", bufs=1, name="psf")
                for hp in range(4):
                    nc.tensor.matmul(
                        ps, lhsT=ach[:, hp, sb * 128:(sb + 1) * 128],
                        rhs=wo[:, hp, n * CH:(n + 1) * CH],
                        start=(hp == 0), stop=(hp == 3))
                nc.vector.tensor_copy(osb, ps)
                nc.sync.dma_start(
                    out=out[sblk * 128:(sblk + 1) * 128,
                            n * CH:(n + 1) * CH], in_=osb)
            return f
        return [oblk(sb, n) for sb in range(4) for n in range(2)]

    pending_norm = []

    def _norm_one(at_ps, dst):
        # Normalize a finished head: recip of the denominator row, rank-1
        # PE broadcast across the 64 dk partitions, multiply into attnT.
        rc = rpool.tile([1, CH], F32R, name="rc", tag="rc")
        with nc.allow_low_precision(reason="f32r feed for PE broadcast"):
            nc.vector.reciprocal(out=rc, in_=at_ps[DK:DK + 1, :])
        bc = pp.tile([DK, CH], F32, tag="at", bufs=3, name="bc")
        nc.tensor.matmul(bc, lhsT=ones, rhs=rc, start=True, stop=True)
        bcs = rpool.tile([DK, CH], F32, tag="bcs", bufs=1, name="bcs")
        nc.vector.tensor_copy(bcs, bc)
        nc.vector.tensor_mul(dst, at_ps[0:DK, :], bcs)

    from collections import deque
    fillers = deque()  # entries: (deadline_head, fn)
    qch, f0 = qkv_fillers(0, xch0)
    for f in f0:
        f()  # nothing to overlap with at the very start

    prev = None  # (j, ach) of the chunk awaiting its O-projection
    for j in range(NCH):
        # stage next chunk's x DMAs + projection fillers, and the previous
        # chunk's O-projection, to fill PE gaps in this ACT-bound phase
        if j + 1 < NCH:
            xch_n = dma_x(j + 1)
            qch_n, fs = qkv_fillers(j + 1, xch_n)
            fillers.extend((None, f) for f in fs)
        else:
            qch_n = None
        if prev is not None:
            fillers.extend((None, f) for f in o_fillers(*prev))

        ach = apool.tile([128, 4, CH], F32R, name=f"ach{j}", tag="ach")
        nkb = 4 * (j + 1)
        steps = HPC * (nkb // 2)
        npop = 0
        nfill0 = len(fillers)
        gstep = 0

        closed = set()

        def emit_pv(ent):
            at_ps, h, pg, pe, is_last = ent
            for t in range(2):
                i = 2 * pg + t
                nc.tensor.matmul(
                    at_ps, lhsT=v[:, i, h, :],
                    rhs=pe[:, t * CH:(t + 1) * CH],
                    start=(i == 0), stop=(is_last and t == 1),
                    skip_group_check=True)
            if is_last:
                closed.add(at_ps.tensor.name)

        def flush_ready():
            # emit norms only for heads whose accumulation group is closed
            # (emission order defines read/write semantics under Tile)
            while pending_norm and pending_norm[0][0].tensor.name in closed:
                at_ps, dst = pending_norm.pop(0)
                _norm_one(at_ps, dst)

        pend = []
        for h in range(HPC):
            while fillers and fillers[0][0] is not None and fillers[0][0] <= h:
                npop += 1
                fillers.popleft()[1]()
            mb, half = h // 2, h % 2
            row = slice(half * DK, (half + 1) * DK)
            at_ps = pp.tile([DK + 1, CH], F32, tag="at", bufs=3, name="at_ps")
            for g in range(nkb // 2):
                i0 = 2 * g
                sc = pp.tile([128, 2 * CH], F32, tag="sc", name="sc")
                for t in range(2):
                    i = i0 + t
                    nc.tensor.matmul(
                        sc[:, t * CH:(t + 1) * CH],
                        lhsT=kT[row, mb, i * 128:(i + 1) * 128],
                        rhs=qch[row, mb, :], start=True, stop=True)
                e = epool.tile([128, 2 * CH], F32R, name="e", tag="e")
                nc.scalar.activation(out=e, in_=sc, func=EXP, scale=0.125)
                for t in range(2):
                    i = i0 + t
                    if i >= 4 * j:
                        # diagonal block: zero entries where k > q
                        nc.gpsimd.affine_select(
                            out=e[:, t * CH:(t + 1) * CH],
                            in_=e[:, t * CH:(t + 1) * CH],
                            compare_op=mybir.AluOpType.is_ge,
                            fill=0.0, base=j * CH - i * 128,
                            channel_multiplier=-1, pattern=[[1, CH]])
                gstep += 1
                if fillers and gstep * nfill0 // steps >= npop + 1:
                    npop += 1
                    fillers.popleft()[1]()
                if len(pend) > 1:
                    emit_pv(pend.pop(0))
                flush_ready()
                pend.append((at_ps, h, g, e, g == nkb // 2 - 1))
            pending_norm.append((at_ps, ach[row, mb, :]))
        while pend:
            emit_pv(pend.pop(0))
        flush_ready()
        assert not pending_norm
        while fillers:
            fillers.popleft()[1]()
        prev = (j, ach)
        qch = qch_n

    jf, achf = prev
    for sb in range(4):
        sblk = jf * 4 + sb
        ps = pp.tile([128, 2 * CH], F32, tag="sc", name="ps_of")
        for n in range(2):
            for hp in range(4):
                nc.tensor.matmul(
                    ps[:, n * CH:(n + 1) * CH],
                    lhsT=achf[:, hp, sb * 128:(sb + 1) * 128],
                    rhs=wo[:, hp, n * CH:(n + 1) * CH],
                    start=(hp == 0), stop=(hp == 3))
        for n in range(2):
            osb = opool.tile([128, CH], F32, name="osb", tag="osb")
            nc.vector.tensor_copy(osb, ps[:, n * CH:(n + 1) * CH])
            nc.sync.dma_start(
                out=out[sblk * 128:(sblk + 1) * 128, n * CH:(n + 1) * CH],
                in_=osb)


def _split_excess_waits(nc, max_waits=1):
    # This walrus build rejects instructions carrying more than a couple of
    # sem waits ("Too many sync wait commands").  Engines execute their
    # stream in order, so excess waits can be moved onto nofuse nops placed
    # immediately before the instruction on the same engine.
    ctr = 0
    for blk in nc.m.functions[0].blocks:
        insts = blk.instructions
        out = []
        changed = False
        for inst in insts:
            si = inst.sync_info
            if si is not None and si.on_wait and len(si.on_wait) > max_waits:
                waits = list(si.on_wait)
                extra, keep = waits[:-max_waits], waits[-max_waits:]
                for gi in range(0, len(extra), max_waits):
                    ctr += 1
                    out.append(mybir.InstNoOp(
                        name=f"wsplit_{ctr}",
                        engine=inst.engine,
                        bass_nofuse=True,
                        sync_info=mybir.SyncInfo(
                            on_wait=extra[gi:gi + max_waits], on_update=[]),
                    ))
                inst.sync_info = mybir.SyncInfo(
                    on_wait=keep, on_update=si.on_update)
                changed = True
            out.append(inst)
        if changed:
            insts[:] = out


_CACHE = {}


def _get_nc(split=True):
    if "nc" in _CACHE:
        return _CACHE["nc"]
    tile.TileContext._drain_and_barrier = _drain_and_barrier_split
    nc = bass.Bass("TRN2", target_bir_lowering=False, debug=False)
    xT = nc.dram_tensor("xT", [D, S], F32R, kind="ExternalInput").ap()
    wqT = nc.dram_tensor("wqT", [D, HD], F32R, kind="ExternalInput").ap()
    wkT = nc.dram_tensor("wkT", [D, HD], F32R, kind="ExternalInput").ap()
    wvT = nc.dram_tensor("wvT", [D, HD], F32R, kind="ExternalInput").ap()
    woT = nc.dram_tensor("woT", [HD, D], F32R, kind="ExternalInput").ap()
    out = nc.dram_tensor("out", [S, D], F32, kind="ExternalOutput").ap()
    from contextlib import ExitStack
    with tile.TileContext(nc) as tc, ExitStack() as ctx:
        _build_kernel(ctx, tc, xT, wqT, wkT, wvT, woT, out)
    if split:
        _split_excess_waits(nc)
        _CACHE["nc"] = nc
    return nc


def make_in_maps(x, Wq, Wk, Wv, Wo):
    x = np.asarray(x, np.float32)
    Wq, Wk, Wv, Wo = (np.asarray(w, np.float32) for w in (Wq, Wk, Wv, Wo))
    in_maps = []
    for c in range(8):
        b, hh = c // 2, c % 2
        cols = slice(hh * HD, (hh + 1) * HD)
        in_maps.append({
            "xT": np.ascontiguousarray(x[b].T),
            "wqT": np.ascontiguousarray(Wq[cols, :].T),
            "wkT": np.ascontiguousarray(Wk[cols, :].T),
            "wvT": np.ascontiguousarray(Wv[cols, :].T),
            "woT": np.ascontiguousarray(Wo[:, cols].T),
        })
    return in_maps


def kernel(x, Wq, Wk, Wv, Wo, _trace=False, _trace_kwargs=None):
    nc = _get_nc()
    in_maps = make_in_maps(x, Wq, Wk, Wv, Wo)
    res = run_bass_kernel_spmd(
        nc, in_maps, core_ids=list(range(8)), trace=_trace,
        **(_trace_kwargs or {}))
    outs = [res.results[c]["out"] for c in range(8)]
    full = np.stack([outs[2 * b] + outs[2 * b + 1] for b in range(B)])
    if _trace:
        _CACHE["last_results"] = res
    return full.astype(np.float32)


# revision 29
# speedup vs baseline: 1.0686x; 1.0686x over previous
"""Causal multi-head self-attention on 8 Trainium2 NeuronCores.

Problem: x[4,2048,1024], Wq/Wk/Wv/Wo[1024,1024], H=16 heads, dk=64.
  q = x@Wq.T, k = x@Wk.T, v = x@Wv.T  (per-head causal softmax(q k^T/8) v) @ Wo.T

Sharding: core c handles batch b=c//2 and head-half hh=c%2 (8 heads).
Each core returns a partial output (its 512 attn columns through the
matching 512 rows of Wo.T); the host sums core pairs.

Kernel layouts (all contractions on the partition axis):
  xT  [1024, 2048]  (d, s)      qT/kT [512, 2048] (head*64+dk, s)
  v   [s-block, head, 65]       (col 64 = ones -> softmax denominator)
  scoresT [k, q] blocks; exp on ACT (scale=1/8, no max-subtraction --
  scores are O(1) here); causal = block skip + affine_select zero-fill
  on diagonal blocks; attnT accumulated in PSUM with the ones column
  giving the denominator; normalization via DVE reciprocal + a rank-1
  PE broadcast matmul; O-projection from attnT layout.
Matmul inputs are bitcast to float32r (full-rate on the PE array).
"""

import numpy as np

import concourse.bass as bass
import concourse.mybir as mybir
import concourse.tile as tile
from concourse.bass_utils import run_bass_kernel_spmd
from concourse.vector_clock import ScopedClock, VectorClock

B, S, D, H, DK = 4, 2048, 1024, 16, 64
HPC = H // 2          # heads per core
HD = HPC * DK         # 512 head-dim columns per core
CH = 512              # q-chunk width
NCH = S // CH         # 4
NKB = S // 128        # 16 k-blocks
F32 = mybir.dt.float32
F32R = mybir.dt.float32r
EXP = mybir.ActivationFunctionType.Exp


def _drain_and_barrier_split(self, tick_clock, wait_clock):
    # The stock Tile tail drain attaches every outstanding sem wait to one
    # Drain instruction; this walrus build caps sync waits per instruction
    # and rejects it.  Put each wait on its own SP nop first, then drain
    # with no waits (SP has observed everything by then).
    gc = tick_clock.global_clock
    n = len(gc)
    for proc in range(n):
        t = gc[proc]
        if t == 0:
            continue
        vc = VectorClock([0] * n)
        vc.require_at_least(proc, t)
        nop = self.nc.sync.nop(nofuse=True)
        wait_clock.add_sem_waits(nop.ins, ScopedClock({None: vc}))
    self.nc.sync.drain()
    self.nc.all_engine_barrier()
    assert self.sems is not None
    popped = self.nc._tile_sem_poison_stack.pop()
    assert popped is self._sem_poison
    self.nc.clear_and_free_semaphores(list(self.sems.allocated().values()))
    self.nc.all_engine_barrier()


def _build_kernel(ctx, tc, xT, wqT, wkT, wvT, woT, out):
    nc = tc.nc
    KC = D // 128  # 8 contraction chunks for the projections

    wpool = ctx.enter_context(tc.tile_pool(name="weights", bufs=1))
    kvpool = ctx.enter_context(tc.tile_pool(name="kv", bufs=1))
    xpool = ctx.enter_context(tc.tile_pool(name="x", bufs=1))
    qpool = ctx.enter_context(tc.tile_pool(name="q", bufs=2))
    epool = ctx.enter_context(tc.tile_pool(name="exp", bufs=4))
    apool = ctx.enter_context(tc.tile_pool(name="attn", bufs=2))
    opool = ctx.enter_context(tc.tile_pool(name="osb", bufs=4))
    rpool = ctx.enter_context(tc.tile_pool(name="recip", bufs=2))
    # One PSUM pool, 8 banks: sc 2x[128,1024] (4) + at 3x[65,512] (3) +
    # bc 1x[64,512] (1).  Projection/O-proj groups share the "sc" slots.
    pp = ctx.enter_context(tc.tile_pool(name="pp", bufs=2, space="PSUM"))

    # --- whole-kernel-resident tiles ---
    wq = [wpool.tile([128, HD], F32R, tag=f"wq{kc}", name=f"wq{kc}")
          for kc in range(KC)]
    wk = [wpool.tile([128, HD], F32R, tag=f"wk{kc}", name=f"wk{kc}")
          for kc in range(KC)]
    wv = [wpool.tile([128, HD], F32R, tag=f"wv{kc}", name=f"wv{kc}")
          for kc in range(KC)]
    wo = wpool.tile([128, 4, D], F32R, tag="wo")
    ones = wpool.tile([1, DK], F32R, tag="ones")
    kT = kvpool.tile([128, 4, S], F32R, tag="kT")
    v = kvpool.tile([128, NKB, HPC, DK + 1], F32R, tag="v")

    def dma_x(j):
        cs = slice(j * CH, (j + 1) * CH)
        xch = [xpool.tile([128, CH], F32R, tag=f"x{kc}", name=f"x{kc}")
               for kc in range(KC)]
        for kc in range(KC):
            nc.sync.dma_start(out=xch[kc], in_=xT[kc * 128:(kc + 1) * 128, cs])
        return xch

    def dma_w(w, wT):
        for kc in range(KC):
            nc.sync.dma_start(out=w[kc], in_=wT[kc * 128:(kc + 1) * 128, :])

    # First matmul needs only xch0[0] + wq[0]: interleave those DMAs first.
    cs0 = slice(0, CH)
    xch0 = [xpool.tile([128, CH], F32R, tag=f"x{kc}", name=f"x{kc}")
            for kc in range(KC)]
    for kc in range(KC):
        nc.sync.dma_start(out=xch0[kc], in_=xT[kc * 128:(kc + 1) * 128, cs0])
        nc.gpsimd.dma_start(out=wq[kc], in_=wqT[kc * 128:(kc + 1) * 128, :])
    dma_w(wk, wkT)
    dma_w(wv, wvT)
    nc.sync.dma_start(out=wo, in_=woT.rearrange("(c p) n -> p c n", p=128))
    ones_f32 = wpool.tile([1, DK], F32, tag="ones_f32")
    nc.vector.memset(ones_f32, 1.0)
    nc.vector.tensor_copy(ones, ones_f32)
    vcol_f32 = wpool.tile([128, NKB, HPC, 1], F32, tag="vcol_f32")
    nc.vector.memset(vcol_f32, 1.0)
    nc.vector.tensor_copy(v[:, :, :, DK:DK + 1], vcol_f32)

    def qkv_fillers(j, xch):
        cs = slice(j * CH, (j + 1) * CH)
        qch = qpool.tile([128, 4, CH], F32R, name=f"qch{j}", tag="qch")
        fillers = []
        dense = j == 0  # attention not running yet: borrow the sc slots

        def proj(w, dst, mb, nmb):
            def f():
                tag = "sc" if dense else "fill"
                ps = pp.tile([128, nmb * CH], F32, tag=tag, bufs=None if dense
                             else 1, name="psf")
                for t in range(nmb):
                    for kc in range(KC):
                        nc.tensor.matmul(
                            ps[:, t * CH:(t + 1) * CH],
                            lhsT=w[kc][:, (mb + t) * 128:(mb + t + 1) * 128],
                            rhs=xch[kc], start=(kc == 0), stop=(kc == KC - 1))
                nc.vector.tensor_copy(dst, ps)
            return f

        def vproj(sb, nsb):
            def f():
                tag = "sc" if dense else "fill"
                ps = pp.tile([128, nsb * CH], F32, tag=tag, bufs=None if dense
                             else 1, name="psf")
                for t in range(nsb):
                    for kc in range(KC):
                        nc.tensor.matmul(
                            ps[:, t * CH:(t + 1) * CH],
                            lhsT=xch[kc][:, (sb + t) * 128:(sb + t + 1) * 128],
                            rhs=wv[kc], start=(kc == 0), stop=(kc == KC - 1))
                sblk = j * 4 + sb
                nc.vector.tensor_copy(
                    v[:, sblk:sblk + nsb, :, 0:DK],
                    ps.rearrange("p (t h d) -> p t h d", t=nsb, h=HPC))
            return f

        nm = 2 if dense else 1
        for mb in range(0, 4, nm):
            fillers.append(proj(wq, qch[:, mb:mb + nm, :], mb, nm))
        for mb in range(0, 4, nm):
            fillers.append(proj(wk, kT[:, mb:mb + nm, cs], mb, nm))
        for sb in range(0, 4, nm):
            fillers.append(vproj(sb, nm))
        return qch, fillers

    def o_fillers(j, ach):
        def oblk(sb, n):
            def f():
                sblk = j * 4 + sb
                osb = opool.tile([128, CH], F32, name="osb", tag="osb")
                ps = pp.tile([128, CH], F32, tag="fill", bufs=1, name="psf")
                for hp in range(4):
                    nc.tensor.matmul(
                        ps, lhsT=ach[:, hp, sb * 128:(sb + 1) * 128],
                        rhs=wo[:, hp, n * CH:(n + 1) * CH],
                        start=(hp == 0), stop=(hp == 3))
                nc.vector.tensor_copy(osb, ps)
                nc.sync.dma_start(
                    out=out[sblk * 128:(sblk + 1) * 128,
                            n * CH:(n + 1) * CH], in_=osb)
            return f
        return [oblk(sb, n) for sb in range(4) for n in range(2)]

    pending_norm = []

    def _norm_one(at_ps, dst):
        # Normalize a finished head: recip of the denominator row, rank-1
        # PE broadcast across the 64 dk partitions, multiply into attnT.
        rc = rpool.tile([1, CH], F32R, name="rc", tag="rc")
        with nc.allow_low_precision(reason="f32r feed for PE broadcast"):
            nc.vector.reciprocal(out=rc, in_=at_ps[DK:DK + 1, :])
        bc = pp.tile([DK, CH], F32, tag="at", bufs=3, name="bc")
        nc.tensor.matmul(bc, lhsT=ones, rhs=rc, start=True, stop=True)
        bcs = rpool.tile([DK, CH], F32, tag="bcs", bufs=1, name="bcs")
        nc.vector.tensor_copy(bcs, bc)
        nc.vector.tensor_mul(dst, at_ps[0:DK, :], bcs)

    from collections import deque
    fillers = deque()  # entries: (deadline_head, fn)
    qch, f0 = qkv_fillers(0, xch0)
    for f in f0:
        f()  # nothing to overlap with at the very start

    prev = None  # (j, ach) of the chunk awaiting its O-projection
    for j in range(NCH):
        # stage next chunk's x DMAs + projection fillers, and the previous
        # chunk's O-projection, to fill PE gaps in this ACT-bound phase
        if j + 1 < NCH:
            xch_n = dma_x(j + 1)
            qch_n, fs = qkv_fillers(j + 1, xch_n)
            fillers.extend((None, f) for f in fs)
        else:
            qch_n = None
        if prev is not None:
            fillers.extend((None, f) for f in o_fillers(*prev))

        ach = apool.tile([128, 4, CH], F32R, name=f"ach{j}", tag="ach")
        nkb = 4 * (j + 1)
        steps = HPC * (nkb // 2)
        npop = 0
        nfill0 = len(fillers)
        gstep = 0

        closed = set()

        def emit_pv(ent):
            at_ps, h, pg, pe, is_last = ent
            for t in range(2):
                i = 2 * pg + t
                if i < 4 * j:
                    ql = 0
                else:
                    ql = min(128 * (i - 4 * j), CH - 256)
                nc.tensor.matmul(
                    at_ps[:, ql:], lhsT=v[:, i, h, :],
                    rhs=pe[:, t * CH + ql:(t + 1) * CH],
                    start=(i == 0), stop=(is_last and t == 1),
                    skip_group_check=True)
            if is_last:
                closed.add(at_ps.tensor.name)

        def flush_ready():
            # emit norms only for heads whose accumulation group is closed
            # (emission order defines read/write semantics under Tile)
            while pending_norm and pending_norm[0][0].tensor.name in closed:
                at_ps, dst = pending_norm.pop(0)
                _norm_one(at_ps, dst)

        pend = []
        for h in range(HPC):
            while fillers and fillers[0][0] is not None and fillers[0][0] <= h:
                npop += 1
                fillers.popleft()[1]()
            mb, half = h // 2, h % 2
            row = slice(half * DK, (half + 1) * DK)
            at_ps = pp.tile([DK + 1, CH], F32, tag="at", bufs=3, name="at_ps")
            for g in range(nkb // 2):
                i0 = 2 * g
                # Diagonal blocks are mostly masked: columns [0, qlo) of
                # k-block i are causally dead (q < k for the whole block),
                # so trim score/exp-mask/PV work to [qlo, CH).  fp32r
                # matmuls below N=256 run at 1/4 rate, so never trim
                # narrower than 256.
                def _qlo(i):
                    if i < 4 * j:
                        return 0
                    return min(128 * (i - 4 * j), CH - 256)

                sc = pp.tile([128, 2 * CH], F32, tag="sc", name="sc")
                pair_ql = _qlo(i0)  # uniform over the pair so the single
                # exp below reads only written PSUM
                for t in range(2):
                    i = i0 + t
                    nc.tensor.matmul(
                        sc[:, t * CH + pair_ql:(t + 1) * CH],
                        lhsT=kT[row, mb, i * 128:(i + 1) * 128],
                        rhs=qch[row, mb, pair_ql:], start=True, stop=True)
                e = epool.tile([128, 2 * CH], F32R, name="e", tag="e")
                sc_v = sc.rearrange("p (t c) -> p t c", t=2)[:, :, pair_ql:]
                e_v = e.rearrange("p (t c) -> p t c", t=2)[:, :, pair_ql:]
                nc.scalar.activation(out=e_v, in_=sc_v, func=EXP, scale=0.125)
                for t in range(2):
                    i = i0 + t
                    if i >= 4 * j:
                        ql = _qlo(i)
                        # zero remaining k > q entries
                        nc.gpsimd.affine_select(
                            out=e[:, t * CH + ql:(t + 1) * CH],
                            in_=e[:, t * CH + ql:(t + 1) * CH],
                            compare_op=mybir.AluOpType.is_ge,
                            fill=0.0, base=j * CH - i * 128 + ql,
                            channel_multiplier=-1, pattern=[[1, CH - ql]])
                gstep += 1
                if fillers and gstep * nfill0 // steps >= npop + 1:
                    npop += 1
                    fillers.popleft()[1]()
                if len(pend) > 1:
                    emit_pv(pend.pop(0))
                flush_ready()
                pend.append((at_ps, h, g, e, g == nkb // 2 - 1))
            pending_norm.append((at_ps, ach[row, mb, :]))
        while pend:
            emit_pv(pend.pop(0))
        flush_ready()
        assert not pending_norm
        while fillers:
            fillers.popleft()[1]()
        prev = (j, ach)
        qch = qch_n

    jf, achf = prev
    for sb in range(4):
        sblk = jf * 4 + sb
        ps = pp.tile([128, 2 * CH], F32, tag="sc", name="ps_of")
        for n in range(2):
            for hp in range(4):
                nc.tensor.matmul(
                    ps[:, n * CH:(n + 1) * CH],
                    lhsT=achf[:, hp, sb * 128:(sb + 1) * 128],
                    rhs=wo[:, hp, n * CH:(n + 1) * CH],
                    start=(hp == 0), stop=(hp == 3))
        for n in range(2):
            osb = opool.tile([128, CH], F32, name="osb", tag="osb")
            nc.vector.tensor_copy(osb, ps[:, n * CH:(n + 1) * CH])
            nc.sync.dma_start(
                out=out[sblk * 128:(sblk + 1) * 128, n * CH:(n + 1) * CH],
                in_=osb)


def _split_excess_waits(nc, max_waits=1):
    # This walrus build rejects instructions carrying more than a couple of
    # sem waits ("Too many sync wait commands").  Engines execute their
    # stream in order, so excess waits can be moved onto nofuse nops placed
    # immediately before the instruction on the same engine.
    ctr = 0
    for blk in nc.m.functions[0].blocks:
        insts = blk.instructions
        out = []
        changed = False
        for inst in insts:
            si = inst.sync_info
            if si is not None and si.on_wait and len(si.on_wait) > max_waits:
                waits = list(si.on_wait)
                extra, keep = waits[:-max_waits], waits[-max_waits:]
                for gi in range(0, len(extra), max_waits):
                    ctr += 1
                    out.append(mybir.InstNoOp(
                        name=f"wsplit_{ctr}",
                        engine=inst.engine,
                        bass_nofuse=True,
                        sync_info=mybir.SyncInfo(
                            on_wait=extra[gi:gi + max_waits], on_update=[]),
                    ))
                inst.sync_info = mybir.SyncInfo(
                    on_wait=keep, on_update=si.on_update)
                changed = True
            out.append(inst)
        if changed:
            insts[:] = out


_CACHE = {}


def _get_nc(split=True):
    if "nc" in _CACHE:
        return _CACHE["nc"]
    tile.TileContext._drain_and_barrier = _drain_and_barrier_split
    nc = bass.Bass("TRN2", target_bir_lowering=False, debug=False)
    xT = nc.dram_tensor("xT", [D, S], F32R, kind="ExternalInput").ap()
    wqT = nc.dram_tensor("wqT", [D, HD], F32R, kind="ExternalInput").ap()
    wkT = nc.dram_tensor("wkT", [D, HD], F32R, kind="ExternalInput").ap()
    wvT = nc.dram_tensor("wvT", [D, HD], F32R, kind="ExternalInput").ap()
    woT = nc.dram_tensor("woT", [HD, D], F32R, kind="ExternalInput").ap()
    out = nc.dram_tensor("out", [S, D], F32, kind="ExternalOutput").ap()
    from contextlib import ExitStack
    with tile.TileContext(nc) as tc, ExitStack() as ctx:
        _build_kernel(ctx, tc, xT, wqT, wkT, wvT, woT, out)
    if split:
        _split_excess_waits(nc)
        _CACHE["nc"] = nc
    return nc


def make_in_maps(x, Wq, Wk, Wv, Wo):
    x = np.asarray(x, np.float32)
    Wq, Wk, Wv, Wo = (np.asarray(w, np.float32) for w in (Wq, Wk, Wv, Wo))
    in_maps = []
    for c in range(8):
        b, hh = c // 2, c % 2
        cols = slice(hh * HD, (hh + 1) * HD)
        in_maps.append({
            "xT": np.ascontiguousarray(x[b].T),
            "wqT": np.ascontiguousarray(Wq[cols, :].T),
            "wkT": np.ascontiguousarray(Wk[cols, :].T),
            "wvT": np.ascontiguousarray(Wv[cols, :].T),
            "woT": np.ascontiguousarray(Wo[:, cols].T),
        })
    return in_maps


def kernel(x, Wq, Wk, Wv, Wo, _trace=False, _trace_kwargs=None):
    nc = _get_nc()
    in_maps = make_in_maps(x, Wq, Wk, Wv, Wo)
    res = run_bass_kernel_spmd(
        nc, in_maps, core_ids=list(range(8)), trace=_trace,
        **(_trace_kwargs or {}))
    outs = [res.results[c]["out"] for c in range(8)]
    full = np.stack([outs[2 * b] + outs[2 * b + 1] for b in range(B)])
    if _trace:
        _CACHE["last_results"] = res
    return full.astype(np.float32)
